# revision 17
# baseline (speedup 1.0000x reference)
"""4-layer GCN (out = adj @ (h @ W) + b, stacked) on 8 trn2 NeuronCores.

Strategy (row-parallel over nodes, fp8 adjacency):
  - Each core owns R = N/8 rows of adj (its output rows for every layer).
  - The PE contracts over the partition dim, so adj tiles are fed k-major
    (transposed).  Pass 0 loads natural f32 adj tiles, scale-casts to
    fp8e4 (x8192 so the ~1/N entries land in fp8's normal range),
    transposes 128x128 tiles on the PE, uses them for the layer-0 GEMM,
    and stores the transposed fp8 adjacency (as [k-block pair] tiles
    [128, 2, R]) to a DRAM scratch — except for CACHE_KB k-blocks that
    stay resident in SBUF for layers 1..3 (skipping both the store and
    all three re-loads).
  - The per-layer dense operand Z_l = h_l @ W_l is computed on the
    OWNING core only ([R, d] rows) and AllGather'd natural row-major in
    N_GATHER_CHUNKS pieces so the next layer's GEMM starts as soon as
    the first piece lands.  Z_1 tolerates fp8 (x16384), so layer 1 runs
    fp8 x fp8 in DoubleRow perf mode (2 k-blocks/matmul, 2x PE rate);
    Z_0 / Z_2 / Z_3 stay bf16 (fp8 there fails the accuracy gate) and
    those layers run bf16-stationary x fp8-moving.
  - Accumulation is fp32 in PSUM; the boundary activation applies
    1/(8192*z_scale) and the bias in one op, keeping h in bf16.

kernel(**inputs) takes the full-size numpy inputs and returns the full
[N, 16] float32 output.
"""

import os

import numpy as np
import ml_dtypes

P = 128            # SBUF partitions / PE tile size
N_CORES = 8
SEG = 512          # fp32 PSUM bank width (free-dim elements)

# Full-problem config (must match the harness problem)
FULL_N = 16384
FULL_D_IN = 128
FULL_D_HID = 64
FULL_N_CLASSES = 16
FULL_N_HIDDEN_LAYERS = 2

ADJ_SCALE = 8192.0   # adj entries ~U[0, 1/N]; x8192 -> [0, 0.5], fp8-normal
Z1_SCALE = 16384.0   # z1 rms ~1.5e-3; x16384 centers it in fp8's range

CACHE_KB = 64        # k-blocks of transposed fp8 adj kept SBUF-resident
N_STRIP_BUFS = 6
N_GATHER_CHUNKS = 2  # pieces per inter-layer Z AllGather

_CACHE = {}
_LAST_RESULTS = None  # BassKernelResults of the most recent run (for test.py)


def _split_dma_waits(nc, mybir, max_waits=1, noop_waits=1):
    """Walrus' DMA pseudo-instruction supports at most 2 sem waits; Tile can
    emit 3+.  Hoist all waits of offending DMAs onto a NoOp on the issuing
    engine immediately before the DMA (same NX stream, so ordering holds)."""
    for f in nc.m.functions:
        for bb in f.blocks:
            insts = bb.instructions
            i = 0
            while i < len(insts):
                ins = insts[i]
                si = ins.sync_info
                if (
                    si is not None
                    and si.on_wait
                    and len(si.on_wait) > max_waits
                ):
                    waits = list(si.on_wait)
                    keep = waits[-max_waits:]
                    extra = waits[:-max_waits]
                    for j in range(0, len(extra), noop_waits):
                        noop = mybir.InstNoOp(
                            name=nc.get_next_instruction_name(),
                            engine=ins.engine,
                            ins=[],
                            outs=[],
                            sync_info=mybir.SyncInfo(
                                on_wait=extra[j : j + noop_waits], on_update=[]
                            ),
                        )
                        insts.insert(i, noop)
                        i += 1
                    ins.sync_info = mybir.SyncInfo(
                        on_wait=keep, on_update=list(si.on_update or [])
                    )
                i += 1


def _build(N, R, layer_dims):
    """Build the per-core Bass program.

    N: total nodes; R: rows per core; layer_dims: [(d_in, d_out), ...]
    """
    import concourse.bass as bass
    import concourse.mybir as mybir
    from concourse import tile, masks

    f32 = mybir.dt.float32
    bf16 = mybir.dt.bfloat16
    fp8 = mybir.dt.float8e4
    Copy = mybir.ActivationFunctionType.Copy
    Ident = mybir.ActivationFunctionType.Identity
    DR = mybir.MatmulPerfMode.DoubleRow

    KB = N // P                    # contraction k-blocks (128)
    NPAIR = KB // 2
    TR = R // P                    # 128-col tiles per strip (16)
    seg_w = min(SEG, R)
    n_seg = R // seg_w             # psum segments of the [do, R] output (4)
    tps = seg_w // P               # transpose tiles per segment (4)
    n_layers = len(layer_dims)
    d_in0 = layer_dims[0][0]
    d_last = layer_dims[-1][1]
    n_ch = N_GATHER_CHUNKS
    kb_per_core = TR               # 16 k-blocks owned per core
    kb_per_ch = kb_per_core // n_ch
    z_scales = [1.0, Z1_SCALE, 1.0, 1.0]
    z_fp8 = [False, True, False, False]

    nc = bass.Bass(trn_type="TRN2", num_devices=N_CORES)

    adj_d = nc.dram_tensor("adj_shard", [R, N], f32, kind="ExternalInput")
    xT_d = nc.dram_tensor("xT", [d_in0, N], bf16, kind="ExternalInput")
    w_d = [
        nc.dram_tensor(f"w{l}", [di, do], bf16, kind="ExternalInput")
        for l, (di, do) in enumerate(layer_dims)
    ]
    b_d = [
        nc.dram_tensor(f"b{l}", [do, 1], f32, kind="ExternalInput")
        for l, (di, do) in enumerate(layer_dims)
    ]
    outT_d = nc.dram_tensor("outT", [d_last, R], f32, kind="ExternalOutput")

    # Layer >= 1 k-block-pair order: grouped by gather-chunk availability.
    # Chunk ch gathers rows [c*R + ch*R/n_ch, ...) of every core c, i.e.
    # k-blocks c*TR + ch*kb_per_ch + s (pairs stay aligned: kb_per_ch even).
    def chunk_pairs(ch):
        return [
            (c * kb_per_core + ch * kb_per_ch) // 2 + s
            for c in range(N_CORES)
            for s in range(kb_per_ch // 2)
        ]

    pair_order1 = [p for ch in range(n_ch) for p in chunk_pairs(ch)]
    cache_pairs = set(pair_order1[: CACHE_KB // 2])

    with tile.TileContext(nc) as tc:
        with (
            tc.tile_pool(name="const", bufs=1) as constp,
            tc.tile_pool(name="xt", bufs=2) as xtp,
            tc.tile_pool(name="z0", bufs=3) as z0p,
            tc.tile_pool(name="zl", bufs=2) as zlp,
            tc.tile_pool(name="nat", bufs=2) as natp,
            tc.tile_pool(name="natb", bufs=2) as natbp,
            tc.tile_pool(name="strip", bufs=N_STRIP_BUFS) as stripp,
            tc.tile_pool(name="cache", bufs=1) as cachep,
            tc.tile_pool(name="h", bufs=2) as hp,
            tc.tile_pool(name="pz", bufs=2, space="PSUM") as pzp,
            tc.tile_pool(name="pt", bufs=2, space="PSUM") as ptp,
            tc.tile_pool(name="ph", bufs=1, space="PSUM") as php,
            tc.tile_pool(name="dram", bufs=1, space="DRAM") as dramp,
        ):
            ident = constp.tile([P, P], fp8, tag="ident")
            masks.make_identity(nc, ident[:])

            w_sb, b_sb = [], []
            for l, (di, do) in enumerate(layer_dims):
                w = constp.tile([di, do], bf16, tag=f"w{l}")
                nc.sync.dma_start(w[:], w_d[l][:])
                b = constp.tile([do, 1], f32, tag=f"b{l}")
                nc.sync.dma_start(b[:], b_d[l][:])
                w_sb.append(w)
                b_sb.append(b)

            # Transposed fp8 adjacency scratch for the streamed pairs.
            adjT = dramp.tile([N, R], fp8, tag="adjT")
            cache_tiles = {}       # pair -> SBUF tile [P, 2, R] fp8

            cc = {}                # (layer, chunk) -> gathered Z DRAM tensor

            for l in range(n_layers):
                di, do = layer_dims[l]
                last = l == n_layers - 1

                ph = php.tile([do, R], f32, tag="ph")

                if l == 0:
                    pair_order = list(range(NPAIR))
                else:
                    pair_order = pair_order1

                # For l >= 1, Z arrives as gathered DRAM pieces; load it in
                # per-(chunk, core) batches of kb_per_ch k-blocks.
                zbatch = {}        # (ch, c) -> SBUF tile [P, kb_per_ch, do]
                zdt = fp8 if z_fp8[l] else bf16

                def zpair_of(pair, l=l, do=do, zdt=zdt, zbatch=zbatch):
                    """Return (tile, mslice) holding k-blocks (2p, 2p+1)."""
                    c, s = divmod(2 * pair, kb_per_core)
                    ch, s2 = divmod(s, kb_per_ch)
                    key = (ch, c)
                    if key not in zbatch:
                        zg = cc[(l - 1, ch)]
                        zt = zlp.tile([P, kb_per_ch, do], zdt, tag="zl")
                        row = c * kb_per_ch * P
                        # ACT-issued: keeps the collective-gated wait off
                        # the SP queue that streams adjT strips.
                        nc.scalar.dma_start(
                            zt[:],
                            zg[row : row + kb_per_ch * P, :].rearrange(
                                "(m p) d -> p m d", p=P
                            ),
                        )
                        zbatch[key] = zt
                    return zbatch[key], s2

                # ---- big GEMM over k-block pairs ------------------------
                for idx, pair in enumerate(pair_order):
                    start = idx == 0
                    stop = idx == NPAIR - 1
                    if l == 0:
                        # z0 for this pair, JIT from streamed x^T blocks
                        blk = pair // 4
                        if pair % 4 == 0:
                            xtb = xtp.tile([di, 8 * P], bf16, tag="xtb")
                            nc.sync.dma_start(
                                xtb[:],
                                xT_d[:, blk * 8 * P : (blk + 1) * 8 * P],
                            )
                            xtb_cur = xtb
                        zt = z0p.tile([P, 2, do], bf16, tag="z0")
                        for j in range(2):
                            mm = (pair % 4) * 2 + j
                            pz = pzp.tile([P, do], f32, tag="pz")
                            nc.tensor.matmul(
                                pz[:],
                                xtb_cur[:, mm * P : (mm + 1) * P],
                                w_sb[0][:],
                                start=True,
                                stop=True,
                            )
                            nc.any.tensor_copy(zt[:, j, :], pz[:])

                        if pair in cache_pairs:
                            strip = cachep.tile([P, 2, R], fp8, tag=f"ck{pair}")
                            cache_tiles[pair] = strip
                        else:
                            strip = stripp.tile([P, 2, R], fp8, tag="strip")
                        for j in range(2):
                            kb = 2 * pair + j
                            nat = natp.tile([P, TR, P], f32, tag="nat")
                            nc.sync.dma_start(
                                nat[:],
                                adj_d[:, kb * P : (kb + 1) * P].rearrange(
                                    "(t p) k -> p t k", p=P
                                ),
                            )
                            natb = natbp.tile([P, TR, P], fp8, tag="natb")
                            h1 = TR // 2
                            nc.scalar.activation(
                                natb[:, :h1], nat[:, :h1], Copy,
                                scale=ADJ_SCALE,
                            )
                            nc.vector.tensor_scalar_mul(
                                natb[:, h1:], nat[:, h1:], ADJ_SCALE
                            )
                            for s in range(n_seg):
                                # fp8 transpose results must land with
                                # element step 2 in PSUM (hw constraint);
                                # component 0 holds the data.
                                pt = ptp.tile([P, seg_w, 2], fp8, tag="pt")
                                for t in range(tps):
                                    nc.tensor.matmul(
                                        pt[:, t * P : (t + 1) * P, 0],
                                        natb[:, s * tps + t, :],
                                        ident[:],
                                        is_transpose=True,
                                        start=(t == 0),
                                        stop=(t == tps - 1),
                                    )
                                nc.any.tensor_copy(
                                    strip[:, j, s * seg_w : (s + 1) * seg_w],
                                    pt[:, :, 0],
                                )
                        if pair not in cache_pairs:
                            # SWDGE-issued: Pool is idle during layer 0, so
                            # the store's waits stay off the SP/ACT queues.
                            nc.gpsimd.dma_start(
                                adjT[2 * pair * P : (2 * pair + 2) * P, :]
                                .rearrange("(j k) i -> k j i", k=P),
                                strip[:],
                            )
                        for s in range(n_seg):
                            for j in range(2):
                                nc.tensor.matmul(
                                    ph[:, s * seg_w : (s + 1) * seg_w],
                                    zt[:, j, :],
                                    strip[:, j, s * seg_w : (s + 1) * seg_w],
                                    start=(start and j == 0),
                                    stop=(stop and j == 1),
                                )
                    else:
                        if pair in cache_pairs:
                            strip = cache_tiles[pair]
                        else:
                            strip = stripp.tile([P, 2, R], fp8, tag="strip")
                            nc.sync.dma_start(
                                strip[:],
                                adjT[2 * pair * P : (2 * pair + 2) * P, :]
                                .rearrange("(j k) i -> k j i", k=P),
                            )
                        zt, s2 = zpair_of(pair)
                        if z_fp8[l]:
                            # fp8 x fp8: DoubleRow, 2 k-blocks per matmul
                            for s in range(n_seg):
                                nc.tensor.matmul(
                                    ph[:, s * seg_w : (s + 1) * seg_w],
                                    zt[:, s2 : s2 + 2, :],
                                    strip[:, :, s * seg_w : (s + 1) * seg_w],
                                    perf_mode=DR,
                                    start=start,
                                    stop=stop,
                                )
                        else:
                            for s in range(n_seg):
                                for j in range(2):
                                    nc.tensor.matmul(
                                        ph[:, s * seg_w : (s + 1) * seg_w],
                                        zt[:, s2 + j, :],
                                        strip[
                                            :, j, s * seg_w : (s + 1) * seg_w
                                        ],
                                        start=(start and j == 0),
                                        stop=(stop and j == 1),
                                    )

                # ---- boundary: h_{l+1} = psum/(8192*z_scale) + b --------
                out_scale = 1.0 / (ADJ_SCALE * z_scales[l])
                if last:
                    hf = hp.tile([do, R], f32, tag="hf")
                    for s in range(n_seg):
                        nc.scalar.activation(
                            hf[:, s * seg_w : (s + 1) * seg_w],
                            ph[:, s * seg_w : (s + 1) * seg_w],
                            Ident,
                            bias=b_sb[l][:, 0:1],
                            scale=out_scale,
                        )
                    nc.sync.dma_start(outT_d[:], hf[:])
                else:
                    hb = hp.tile([do, R], bf16, tag="hb")
                    for s in range(n_seg):
                        nc.scalar.activation(
                            hb[:, s * seg_w : (s + 1) * seg_w],
                            ph[:, s * seg_w : (s + 1) * seg_w],
                            Ident,
                            bias=b_sb[l][:, 0:1],
                            scale=out_scale,
                        )
                    # local Z_{l+1} rows = hb @ W_{l+1}, gathered in n_ch
                    # row-pieces so the next layer starts early.
                    dn = layer_dims[l + 1][1]
                    ndt = fp8 if z_fp8[l + 1] else bf16
                    nscale = z_scales[l + 1]
                    for ch in range(n_ch):
                        zloc = hp.tile([P, kb_per_ch, dn], ndt, tag="zloc")
                        for m in range(kb_per_ch):
                            mb = ch * kb_per_ch + m
                            pzn = pzp.tile([P, dn], f32, tag="pz")
                            nc.tensor.matmul(
                                pzn[:],
                                hb[:, mb * P : (mb + 1) * P],
                                w_sb[l + 1][:],
                                start=True,
                                stop=True,
                            )
                            if nscale != 1.0:
                                nc.scalar.activation(
                                    zloc[:, m, :], pzn[:], Copy, scale=nscale
                                )
                            else:
                                nc.any.tensor_copy(zloc[:, m, :], pzn[:])
                        cc_in = dramp.tile(
                            [R // n_ch, dn], ndt, tag=f"ccin{l}_{ch}"
                        )
                        nc.scalar.dma_start(
                            cc_in[:].rearrange("(m p) d -> p m d", p=P),
                            zloc[:],
                        )
                        cc_out = dramp.tile(
                            [N // n_ch, dn],
                            ndt,
                            addr_space="Shared",
                            tag=f"ccout{l}_{ch}",
                        )
                        nc.gpsimd.collective_compute(
                            "AllGather",
                            mybir.AluOpType.bypass,
                            replica_groups=[list(range(N_CORES))],
                            ins=[cc_in.opt()],
                            outs=[cc_out.opt()],
                        )
                        cc[(l, ch)] = cc_out
    _split_dma_waits(nc, mybir)
    return nc


def _prep_inputs(x, adj, W_in, b_in, W_hidden, b_hidden, W_out, b_out, N, R):
    bf = ml_dtypes.bfloat16
    xT = np.ascontiguousarray(np.asarray(x, dtype=np.float32).T).astype(bf)
    ws = [np.asarray(W_in)] + [np.asarray(W_hidden)[i] for i in range(np.asarray(W_hidden).shape[0])] + [np.asarray(W_out)]
    bs = [np.asarray(b_in)] + [np.asarray(b_hidden)[i] for i in range(np.asarray(b_hidden).shape[0])] + [np.asarray(b_out)]
    ws = [np.ascontiguousarray(w.astype(np.float32)).astype(bf) for w in ws]
    bs = [np.ascontiguousarray(b.astype(np.float32).reshape(-1, 1)) for b in bs]
    adj = np.asarray(adj, dtype=np.float32)
    in_maps = []
    for c in range(N_CORES):
        m = {"adj_shard": np.ascontiguousarray(adj[c * R : (c + 1) * R]), "xT": xT}
        for l, (w, b) in enumerate(zip(ws, bs)):
            m[f"w{l}"] = w
            m[f"b{l}"] = b
        in_maps.append(m)
    return in_maps


def _run(nc, in_maps, trace=False):
    from concourse.bass_utils import run_bass_kernel_spmd

    global _LAST_RESULTS
    try:
        res = run_bass_kernel_spmd(
            nc, in_maps, core_ids=list(range(N_CORES)), trace=trace
        )
    except ModuleNotFoundError:
        # NTFF profile hook unavailable in this container; rerun untraced.
        res = run_bass_kernel_spmd(
            nc, in_maps, core_ids=list(range(N_CORES)), trace=False
        )
    _LAST_RESULTS = res
    return res.results


def kernel(x, adj, W_in, b_in, W_hidden, b_hidden, W_out, b_out):
    N = FULL_N
    R = N // N_CORES
    layer_dims = (
        [(FULL_D_IN, FULL_D_HID)]
        + [(FULL_D_HID, FULL_D_HID)] * FULL_N_HIDDEN_LAYERS
        + [(FULL_D_HID, FULL_N_CLASSES)]
    )
    key = (N, R, tuple(layer_dims))
    if key not in _CACHE:
        _CACHE[key] = _build(N, R, layer_dims)
    nc = _CACHE[key]
    in_maps = _prep_inputs(
        x, adj, W_in, b_in, W_hidden, b_hidden, W_out, b_out, N, R
    )
    trace = os.environ.get("GCN_TRACE", "0") == "1"
    results = _run(nc, in_maps, trace=trace)
    out = np.empty((N, FULL_N_CLASSES), dtype=np.float32)
    for c in range(N_CORES):
        out[c * R : (c + 1) * R, :] = results[c]["outT"].T
    return out
